# revision 23
# baseline (speedup 1.0000x reference)
"""Conditional contrastive loss on 8 TRN2 NeuronCores (Bass/Tile).

Strategy (data-parallel over rows, per sharding hint):
  - Each core owns 512 rows (of 4096) of inst_embed ("x") and proxy ("p").
  - Host does all O(N*D) prep: row-normalizes x and p (exactly as the
    reference: x / max(||x||, 1e-8)), scales by S=8, quantizes to fp8-e4m3,
    and lays the data out matmul-native (chunk-major [128, KC, n] so the
    tensor engine runs DoubleRow fp8 matmuls with K=256 per instruction).
    Each core's copy of the column matrix is ROTATED so its own 512
    columns come first: row sums are permutation-invariant, and the i2i
    lhsT then aliases xn8[:, :, 0:512] -- no separate x-lhsT load. The
    pre-gathered mask rows negative_mask[labels] get the same column
    permutation (bf16; fp8 masks make the DVE pass slower).
  - Device per core: 16 output tiles ([128 rows, 2048 cols]); each tile is
    8 DoubleRow fp8 matmuls accumulated in PSUM (4 banks), then
    exp((sim - margin)/T) on the scalar engine straight out of PSUM with
    accum_out = row sums -> denominator for free; z to SBUF in bf16;
    numerator = scalar_tensor_tensor(z * mask) on DVE with accum_out
    (walrus rejects TensorScalarPtr on Pool; DVE at 1x is the 2.2us/tile
    steady-state pace-setter).
  - The first tile is split into four 512-column pieces so the
    PE->ACT->DVE pipeline primes ~4us earlier; a short zeros-matmul HAM
    warm-up covers the DMA latency of the first real operands.
  - All load DMAs are issued on the Sync sequencer in consumption order
    (its strict-FIFO DIRECT2D descriptor generation, ~0.65us each, is
    what actually prioritizes the flows).
  - Device ships the raw [128, 16, 8] fp32 accumulator; the host does the
    final O(N) reduce + ln + mean + gather across cores.
"""
import numpy as np
import ml_dtypes

import concourse.bacc as bacc
import concourse.tile as tile
from concourse import mybir, bass_utils

N_FULL = 4096
D = 512
C = 100
N_CORES = 8
RP = N_FULL // N_CORES  # rows per core = 512
P = 128                 # SBUF partitions
KC = D // P             # contraction chunks = 4
NPAIR = KC // 2         # DoubleRow K-pairs = 2
JT = 512                # columns per PSUM bank
JG = 2048               # columns per PSUM group (4 banks)
NG = N_FULL // JG       # groups per (i-tile, matrix) = 2
IT = RP // P            # i-tiles per core = 4
NSLOT = 8               # accumulator slots per (it, mat, kind)
SCALE = 8.0             # fp8 quantization scale on normalized rows
WARMUP_MM = 7           # zeros matmuls to lift the HAM clock gate

F32 = mybir.dt.float32
BF16 = mybir.dt.bfloat16
FP8 = mybir.dt.float8e4
AF = mybir.ActivationFunctionType
ALU = mybir.AluOpType
AX = mybir.AxisListType
DR = mybir.MatmulPerfMode.DoubleRow

_CACHE = {}


def _pin_act_table_set():
    """This kernel only uses Exp. Left alone, bacc's table chooser picks
    exp_and_others; pin everything to natural_log_exp_and_others so any
    future Ln shares the one set -> a single load at startup."""
    from concourse import hw_specs
    orig = hw_specs.get_activation_tables

    def patched(arch):
        tabs = orig(arch)
        return {name: (fns if name == "natural_log_exp_and_others" else set())
                for name, fns in tabs.items()}

    bacc.get_activation_tables = patched


def _build(inv_t: float, bias_den: float):
    _pin_act_table_set()
    nc = bacc.Bacc("TRN2", target_bir_lowering=False, debug=False,
                   num_devices=N_CORES)

    # chunk-major fp8, per-core column-rotated: [kp, kc, j] with the core's
    # own 512 columns first
    xn8 = nc.dram_tensor("xn8", [P, KC, N_FULL], FP8, kind="ExternalInput")
    pc8 = nc.dram_tensor("pc8", [P, KC, RP], FP8, kind="ExternalInput")
    mk = nc.dram_tensor("mk", [RP, N_FULL], BF16, kind="ExternalInput")
    out = nc.dram_tensor("out", [P, 4 * IT * NSLOT], F32,
                         kind="ExternalOutput")

    with tile.TileContext(nc) as tc:
        with (
            tc.tile_pool(name="xpool", bufs=1) as xpool,
            tc.tile_pool(name="lhs", bufs=1) as lhs,
            tc.tile_pool(name="zpool", bufs=3) as zpool,
            tc.tile_pool(name="maskp", bufs=1) as maskp,
            tc.tile_pool(name="small", bufs=1) as small,
            tc.tile_pool(name="ps", bufs=2, space="PSUM") as pspool,
        ):
            # ---- constants (no DMA dependency) ----
            zeros_l = small.tile([P, P], BF16, name="zeros_l")
            nc.vector.memset(zeros_l[:], 0.0)
            zeros_r = small.tile([P, JT], BF16, name="zeros_r")
            nc.vector.memset(zeros_r[:], 0.0)
            bias_t = small.tile([P, 1], F32, name="bias_t")
            nc.vector.memset(bias_t[:], bias_den)
            acc = small.tile([P, 4 * IT, NSLOT], F32, name="acc")
            nc.vector.memset(acc[:], 0.0)

            # ---- loads, all on the Sync sequencer, in consumption order
            pc8_t = lhs.tile([P, KC, RP], FP8, name="pc8_t")
            xn8_t = xpool.tile([P, KC, N_FULL], FP8, name="xn8_t")
            mask_t = [maskp.tile([P, N_FULL], BF16, name=f"mask{it}")
                      for it in range(IT)]
            for g in range(NG):
                for pr in range(NPAIR):
                    nc.sync.dma_start(
                        xn8_t[:, 2 * pr:2 * pr + 2, g * JG:(g + 1) * JG],
                        xn8.ap()[:, 2 * pr:2 * pr + 2, g * JG:(g + 1) * JG])
                if g == 0:
                    # pc8 is not needed until the second tile (the first
                    # tile is i2i, whose lhsT aliases xn8 cols 0:512)
                    nc.sync.dma_start(pc8_t[:], pc8.ap())
                for it in range(IT):
                    nc.sync.dma_start(
                        mask_t[it][:, g * JG:(g + 1) * JG],
                        mk.ap()[it * P:(it + 1) * P, g * JG:(g + 1) * JG])

            def acc_col(it, mat, kind, slot):
                c = it * 4 + mat * 2 + kind
                return acc[:, c, slot:slot + 1]

            # zo is write-only scratch (only the STT's accum_out matters);
            # one shared buffer keeps the WAW chain on the already-serial
            # DVE and cuts tile/semaphore count.
            zo = zpool.tile([P, JG], BF16, name="zo", bufs=1)

            # ---- main loop (group-major; i2i first: its lhsT needs no
            # extra load, so the pipeline starts on 0.5MB of DMA) ----
            for g in range(NG):
                for it in range(IT):
                    i0 = it * P
                    for mat in (1, 0):
                        # i2i lhsT aliases the core's own (rotated-first)
                        # columns of xn8
                        lh = pc8_t if mat == 0 else xn8_t
                        ps = pspool.tile([P, JG], F32,
                                         name=f"ps_{it}_{mat}_{g}", tag="ps")
                        first_tile = (g == 0 and it == 0 and mat == 1)
                        if first_tile:
                            # HAM warm-up: accumulate exact zeros into bank 0
                            # starting at t~0 (no DMA dependency). 10 cold
                            # matmuls span ~4.3us: the clock gate lifts and
                            # the warm-up ends right around when the first
                            # real operands' DMA lands.
                            for w in range(WARMUP_MM):
                                nc.tensor.matmul(
                                    ps[:, 0:JT], zeros_l[:], zeros_r[:],
                                    start=(w == 0), stop=False,
                                )
                        for pr in range(NPAIR):
                            for jl in range(JG // JT):
                                j0 = g * JG + jl * JT
                                nc.tensor.matmul(
                                    ps[:, jl * JT:(jl + 1) * JT],
                                    lh[:, 2 * pr:2 * pr + 2, i0:i0 + P],
                                    xn8_t[:, 2 * pr:2 * pr + 2, j0:j0 + JT],
                                    start=(pr == 0 and not (first_tile and jl == 0)),
                                    stop=(pr == NPAIR - 1),
                                    perf_mode=DR,
                                )
                        z = zpool.tile([P, JG], BF16,
                                       name=f"z_{it}_{mat}_{g}", tag="z")
                        nc.scalar.activation(
                            z[:], ps[:], AF.Exp,
                            bias=bias_t[:], scale=inv_t / (SCALE * SCALE),
                            accum_out=acc_col(it, mat, 0, g),
                        )
                        nc.vector.scalar_tensor_tensor(
                            out=zo[:], in0=z[:], scalar=1.0,
                            in1=mask_t[it][:, g * JG:(g + 1) * JG],
                            op0=ALU.mult, op1=ALU.mult,
                            accum_out=acc_col(it, mat, 1, g),
                        )

            # ---- tail: ship the raw accumulator; host reduces ----
            nc.sync.dma_start(out.ap(), acc[:])

    nc.compile()
    return nc


def make_in_maps(x, p, nmf, lab):
    eps = 1e-8
    xn = x / np.maximum(np.linalg.norm(x, axis=-1, keepdims=True), eps)
    pn = p / np.maximum(np.linalg.norm(p, axis=-1, keepdims=True), eps)
    f8 = ml_dtypes.float8_e4m3
    # chunk-major [P, KC, n]: [kp, kc, j] = v[j, kc*P + kp]
    xn8 = np.ascontiguousarray(
        (xn.T * SCALE).astype(f8).reshape(KC, P, N_FULL).transpose(1, 0, 2))
    pn8 = np.ascontiguousarray(
        (pn.T * SCALE).astype(f8).reshape(KC, P, N_FULL).transpose(1, 0, 2))
    in_maps = []
    for c in range(N_CORES):
        rows = slice(c * RP, (c + 1) * RP)
        # rotate columns so the core's own block comes first (row sums are
        # permutation-invariant; the i2i lhsT aliases cols 0:512)
        perm = np.r_[np.arange(c * RP, (c + 1) * RP),
                     np.arange(0, c * RP), np.arange((c + 1) * RP, N_FULL)]
        in_maps.append({
            "xn8": np.ascontiguousarray(xn8[:, :, perm]),
            "pc8": np.ascontiguousarray(pn8[:, :, rows]),
            "mk": nmf[lab[rows]][:, perm].astype(ml_dtypes.bfloat16),
        })
    return in_maps


def kernel(inst_embed, proxy, negative_mask, labels, temperature, margin):
    t = float(np.asarray(temperature))
    m = float(np.asarray(margin))
    inv_t = 1.0 / t
    bias_den = -m / t

    key = (t, m)
    if key not in _CACHE:
        _CACHE[key] = _build(inv_t, bias_den)
    nc = _CACHE[key]

    x = np.asarray(inst_embed, dtype=np.float32)
    p = np.asarray(proxy, dtype=np.float32)
    nmf = np.asarray(negative_mask, dtype=np.float32)
    lab = np.asarray(labels).astype(np.int64)

    in_maps = make_in_maps(x, p, nmf, lab)

    res = bass_utils.run_bass_kernel_spmd(nc, in_maps,
                                          core_ids=list(range(N_CORES)))
    # acc[p, it*4 + mat*2 + kind, slot]; row it*128+p of the core's block
    parts = []
    for c in range(N_CORES):
        a = res.results[c]["out"].reshape(P, 4 * IT, NSLOT)
        sums = a.astype(np.float64).sum(-1)            # [P, 16]
        lns = np.log(sums).reshape(P, IT, 4).transpose(1, 0, 2)
        parts.append(lns.reshape(RP, 4))
    outs = np.concatenate(parts, axis=0)
    ld_p, ln_p, ld_i, ln_i = (outs[:, q] for q in range(4))
    loss = (-2.0 * np.log(t)
            + (ld_p - ln_p).mean()
            + (ld_i - ln_i).mean())
    return np.float32(loss)


# revision 25
# speedup vs baseline: 1.0345x; 1.0345x over previous
"""Conditional contrastive loss on 8 TRN2 NeuronCores (Bass/Tile).

Strategy (data-parallel over rows, per sharding hint):
  - Each core owns 512 rows (of 4096) of inst_embed ("x") and proxy ("p").
  - Host does all O(N*D) prep: row-normalizes x and p (exactly as the
    reference: x / max(||x||, 1e-8)), scales by S=8, quantizes to fp8-e4m3,
    and lays the data out matmul-native (chunk-major [128, KC, n] so the
    tensor engine runs DoubleRow fp8 matmuls with K=256 per instruction).
    Each core's copy of the column matrix is ROTATED so its own 512
    columns come first: row sums are permutation-invariant, and the i2i
    lhsT then aliases xn8[:, :, 0:512] -- no separate x-lhsT load. The
    pre-gathered mask rows negative_mask[labels] get the same column
    permutation (bf16; fp8 masks make the DVE pass slower).
  - Device per core: 16 output tiles ([128 rows, 2048 cols]); each tile is
    8 DoubleRow fp8 matmuls accumulated in PSUM (4 banks), then
    exp((sim - margin)/T) on the scalar engine straight out of PSUM with
    accum_out = row sums -> denominator for free; z to SBUF in bf16;
    numerator = scalar_tensor_tensor(z * mask) on DVE with accum_out
    (walrus rejects TensorScalarPtr on Pool; DVE at 1x is the 2.2us/tile
    steady-state pace-setter).
  - The first tile is split into four 512-column pieces so the
    PE->ACT->DVE pipeline primes ~4us earlier; a short zeros-matmul HAM
    warm-up covers the DMA latency of the first real operands.
  - All load DMAs are issued on the Sync sequencer in consumption order
    (its strict-FIFO DIRECT2D descriptor generation, ~0.65us each, is
    what actually prioritizes the flows).
  - Device ships the raw [128, 16, 8] fp32 accumulator; the host does the
    final O(N) reduce + ln + mean + gather across cores.
"""
import numpy as np
import ml_dtypes

import concourse.bacc as bacc
import concourse.tile as tile
from concourse import mybir, bass_utils

N_FULL = 4096
D = 512
C = 100
N_CORES = 8
RP = N_FULL // N_CORES  # rows per core = 512
P = 128                 # SBUF partitions
KC = D // P             # contraction chunks = 4
NPAIR = KC // 2         # DoubleRow K-pairs = 2
JT = 512                # columns per PSUM bank
JG = 2048               # columns per PSUM group (4 banks)
NG = N_FULL // JG       # groups per (i-tile, matrix) = 2
IT = RP // P            # i-tiles per core = 4
NSLOT = 8               # accumulator slots per (it, mat, kind)
SCALE = 8.0             # fp8 quantization scale on normalized rows
WARMUP_MM = 7           # zeros matmuls to lift the HAM clock gate

F32 = mybir.dt.float32
BF16 = mybir.dt.bfloat16
FP8 = mybir.dt.float8e4
AF = mybir.ActivationFunctionType
ALU = mybir.AluOpType
AX = mybir.AxisListType
DR = mybir.MatmulPerfMode.DoubleRow

_CACHE = {}


def _pin_act_table_set():
    """This kernel only uses Exp. Left alone, bacc's table chooser picks
    exp_and_others; pin everything to natural_log_exp_and_others so any
    future Ln shares the one set -> a single load at startup."""
    from concourse import hw_specs
    orig = hw_specs.get_activation_tables

    def patched(arch):
        tabs = orig(arch)
        return {name: (fns if name == "natural_log_exp_and_others" else set())
                for name, fns in tabs.items()}

    bacc.get_activation_tables = patched


def _build(inv_t: float, bias_den: float):
    _pin_act_table_set()
    nc = bacc.Bacc("TRN2", target_bir_lowering=False, debug=False,
                   num_devices=N_CORES)

    # chunk-major fp8, per-core column-rotated: [kp, kc, j] with the core's
    # own 512 columns first
    xn8 = nc.dram_tensor("xn8", [P, KC, N_FULL], FP8, kind="ExternalInput")
    pc8 = nc.dram_tensor("pc8", [P, KC, RP], FP8, kind="ExternalInput")
    mk = nc.dram_tensor("mk", [RP, N_FULL], BF16, kind="ExternalInput")
    out = nc.dram_tensor("out", [P, 4 * IT * NSLOT], F32,
                         kind="ExternalOutput")

    with tile.TileContext(nc) as tc:
        with (
            tc.tile_pool(name="xpool", bufs=1) as xpool,
            tc.tile_pool(name="lhs", bufs=1) as lhs,
            tc.tile_pool(name="zpool", bufs=3) as zpool,
            tc.tile_pool(name="maskp", bufs=1) as maskp,
            tc.tile_pool(name="small", bufs=1) as small,
            tc.tile_pool(name="ps", bufs=2, space="PSUM") as pspool,
        ):
            # ---- constants (no DMA dependency) ----
            zeros_l = small.tile([P, P], BF16, name="zeros_l")
            nc.vector.memset(zeros_l[:], 0.0)
            zeros_r = small.tile([P, JT], BF16, name="zeros_r")
            nc.vector.memset(zeros_r[:], 0.0)
            bias_t = small.tile([P, 1], F32, name="bias_t")
            nc.vector.memset(bias_t[:], bias_den)
            acc = small.tile([P, 4 * IT, NSLOT], F32, name="acc")
            nc.vector.memset(acc[:], 0.0)

            # ---- loads, all on the Sync sequencer, in consumption order
            pc8_t = lhs.tile([P, KC, RP], FP8, name="pc8_t")
            xn8_t = xpool.tile([P, KC, N_FULL], FP8, name="xn8_t")
            mask_t = [maskp.tile([P, N_FULL], BF16, name=f"mask{it}")
                      for it in range(IT)]
            for g in range(NG):
                for pr in range(NPAIR):
                    nc.sync.dma_start(
                        xn8_t[:, 2 * pr:2 * pr + 2, g * JG:(g + 1) * JG],
                        xn8.ap()[:, 2 * pr:2 * pr + 2, g * JG:(g + 1) * JG])
                if g == 0:
                    # pc8 is not needed until the second tile (the first
                    # tile is i2i, whose lhsT aliases xn8 cols 0:512)
                    nc.sync.dma_start(pc8_t[:], pc8.ap())
                for it in range(IT):
                    nc.sync.dma_start(
                        mask_t[it][:, g * JG:(g + 1) * JG],
                        mk.ap()[it * P:(it + 1) * P, g * JG:(g + 1) * JG])

            def acc_col(it, mat, kind, slot):
                c = it * 4 + mat * 2 + kind
                return acc[:, c, slot:slot + 1]

            # ---- main loop (group-major; i2i first: its lhsT needs no
            # extra load, so the pipeline starts on 0.5MB of DMA) ----
            for g in range(NG):
                for it in range(IT):
                    i0 = it * P
                    for mat in (1, 0):
                        # i2i lhsT aliases the core's own (rotated-first)
                        # columns of xn8
                        lh = pc8_t if mat == 0 else xn8_t
                        ps = pspool.tile([P, JG], F32,
                                         name=f"ps_{it}_{mat}_{g}", tag="ps")
                        first_tile = (g == 0 and it == 0 and mat == 1)
                        if first_tile:
                            # HAM warm-up: accumulate exact zeros into bank 0
                            # starting at t~0 (no DMA dependency). 10 cold
                            # matmuls span ~4.3us: the clock gate lifts and
                            # the warm-up ends right around when the first
                            # real operands' DMA lands.
                            for w in range(WARMUP_MM):
                                nc.tensor.matmul(
                                    ps[:, 0:JT], zeros_l[:], zeros_r[:],
                                    start=(w == 0), stop=False,
                                )
                        for pr in range(NPAIR):
                            for jl in range(JG // JT):
                                j0 = g * JG + jl * JT
                                nc.tensor.matmul(
                                    ps[:, jl * JT:(jl + 1) * JT],
                                    lh[:, 2 * pr:2 * pr + 2, i0:i0 + P],
                                    xn8_t[:, 2 * pr:2 * pr + 2, j0:j0 + JT],
                                    start=(pr == 0 and not (first_tile and jl == 0)),
                                    stop=(pr == NPAIR - 1),
                                    perf_mode=DR,
                                )
                        z = zpool.tile([P, JG], BF16,
                                       name=f"z_{it}_{mat}_{g}", tag="z")
                        # zo is write-only scratch (only accum_out matters),
                        # but it needs 2 rotating buffers: a shared buffer
                        # adds a ~90ns WAW drain stall inside every STT plus
                        # a sem gap between consecutive STTs.
                        zo = zpool.tile([P, JG], BF16,
                                        name=f"zo_{it}_{mat}_{g}", tag="zo",
                                        bufs=2)
                        nc.scalar.activation(
                            z[:], ps[:], AF.Exp,
                            bias=bias_t[:], scale=inv_t / (SCALE * SCALE),
                            accum_out=acc_col(it, mat, 0, g),
                        )
                        nc.vector.scalar_tensor_tensor(
                            out=zo[:], in0=z[:], scalar=1.0,
                            in1=mask_t[it][:, g * JG:(g + 1) * JG],
                            op0=ALU.mult, op1=ALU.mult,
                            accum_out=acc_col(it, mat, 1, g),
                        )

            # ---- tail: ship the raw accumulator; host reduces ----
            nc.sync.dma_start(out.ap(), acc[:])

    nc.compile()
    return nc


def make_in_maps(x, p, nmf, lab):
    eps = 1e-8
    xn = x / np.maximum(np.linalg.norm(x, axis=-1, keepdims=True), eps)
    pn = p / np.maximum(np.linalg.norm(p, axis=-1, keepdims=True), eps)
    f8 = ml_dtypes.float8_e4m3
    # chunk-major [P, KC, n]: [kp, kc, j] = v[j, kc*P + kp]
    xn8 = np.ascontiguousarray(
        (xn.T * SCALE).astype(f8).reshape(KC, P, N_FULL).transpose(1, 0, 2))
    pn8 = np.ascontiguousarray(
        (pn.T * SCALE).astype(f8).reshape(KC, P, N_FULL).transpose(1, 0, 2))
    in_maps = []
    for c in range(N_CORES):
        rows = slice(c * RP, (c + 1) * RP)
        # rotate columns so the core's own block comes first (row sums are
        # permutation-invariant; the i2i lhsT aliases cols 0:512)
        perm = np.r_[np.arange(c * RP, (c + 1) * RP),
                     np.arange(0, c * RP), np.arange((c + 1) * RP, N_FULL)]
        in_maps.append({
            "xn8": np.ascontiguousarray(xn8[:, :, perm]),
            "pc8": np.ascontiguousarray(pn8[:, :, rows]),
            "mk": nmf[lab[rows]][:, perm].astype(ml_dtypes.bfloat16),
        })
    return in_maps


def kernel(inst_embed, proxy, negative_mask, labels, temperature, margin):
    t = float(np.asarray(temperature))
    m = float(np.asarray(margin))
    inv_t = 1.0 / t
    bias_den = -m / t

    key = (t, m)
    if key not in _CACHE:
        _CACHE[key] = _build(inv_t, bias_den)
    nc = _CACHE[key]

    x = np.asarray(inst_embed, dtype=np.float32)
    p = np.asarray(proxy, dtype=np.float32)
    nmf = np.asarray(negative_mask, dtype=np.float32)
    lab = np.asarray(labels).astype(np.int64)

    in_maps = make_in_maps(x, p, nmf, lab)

    res = bass_utils.run_bass_kernel_spmd(nc, in_maps,
                                          core_ids=list(range(N_CORES)))
    # acc[p, it*4 + mat*2 + kind, slot]; row it*128+p of the core's block
    parts = []
    for c in range(N_CORES):
        a = res.results[c]["out"].reshape(P, 4 * IT, NSLOT)
        sums = a.astype(np.float64).sum(-1)            # [P, 16]
        lns = np.log(sums).reshape(P, IT, 4).transpose(1, 0, 2)
        parts.append(lns.reshape(RP, 4))
    outs = np.concatenate(parts, axis=0)
    ld_p, ln_p, ld_i, ln_i = (outs[:, q] for q in range(4))
    loss = (-2.0 * np.log(t)
            + (ld_p - ln_p).mean()
            + (ld_i - ln_i).mean())
    return np.float32(loss)
